# revision 1
# baseline (speedup 1.0000x reference)
"""Trainium2 Bass kernel for a dense transformer block (pre-LN, causal MHA + FF).

Reference semantics (fp32, per batch row b of 2048, seq T=64, embd C=256):
    h   = LN(x; g1, be1)
    q,k,v = per-head projections (16 heads x 32 dims)
    att = softmax(causal(q k^T / sqrt(32))) v        -> concat heads
    x2  = x + att @ Wp + bp
    out = x2 + relu(LN(x2; g2, be2) @ W1 + b1) @ W2 + b2

Strategy: pure data parallel over 8 NeuronCores (256 batch rows each).
Per core: 128 tiles of 128 tokens (2 batch rows per tile). Token-major
layernorm (bn_stats), PE transposes to feature-major for matmuls,
per-head attention with PE tile_position packing, ones-column trick for
the softmax denominator, host-side folding of LN affine params into the
projection weights.
"""

import os
import sys

sys.path.insert(0, "/opt/trn_rl_repo")

import numpy as np
import concourse.bass as bass
import concourse.mybir as mybir
import concourse.tile as tile
from concourse.vector_clock import ScopedClock
from concourse.bass_utils import run_bass_kernel_spmd

# ---------------------------------------------------------------- constants
N_CORES = 8
N_EMBD = 256
N_HEAD = 16
HEAD = 32
T = 64
BATCH = 2048
B_LOC = BATCH // N_CORES          # 256 batch rows per core
TOK = B_LOC * T                   # 16384 tokens per core
P = 128                           # tokens per tile (2 batch rows)
NT = TOK // P                     # 128 tiles per core
SCALE = 1.0 / np.sqrt(HEAD)
EPS = 1e-5

FDT = mybir.dt.float32
CDT = mybir.dt.bfloat16           # matmul operand dtype
NP_CDT = {mybir.dt.bfloat16: "bfloat16", mybir.dt.float32: "float32"}

AFT = mybir.ActivationFunctionType

# walrus (this build) only encodes 1 sync wait on CTRL-class (Drain) insts
_MAX_DRAIN_WAITS = 1


def _split_waits(nc, limit=1):
    """walrus in this build encodes only `limit` sync waits per CTRL/compute
    instruction; move overflow waits onto preceding same-engine NOPs
    (equivalent: the engine blocks at the NOP instead of at the inst).
    DMA instructions are exempt: their waits are consumed asynchronously by
    the DGE descriptor, so moving them onto a blocking sequencer NOP could
    stall the issue queue behind work that produces the awaited sem."""
    n = 0
    for f in nc.m.functions:
        for bb in f.blocks:
            insts = bb.instructions
            i = 0
            while i < len(insts):
                inst = insts[i]
                si = getattr(inst, "sync_info", None)
                if si is not None and si.on_wait and len(si.on_wait) > limit:
                    waits = list(si.on_wait)
                    keep, extra = waits[:limit], waits[limit:]
                    inst.sync_info = mybir.SyncInfo(
                        on_wait=keep, on_update=list(si.on_update or [])
                    )
                    for j, w in enumerate(extra):
                        nop = mybir.InstNoOp(
                            name=f"{inst.name}-wsplit{j}",
                            ins=[], outs=[],
                            engine=inst.engine,
                            bass_nofuse=True,
                            sync_info=mybir.SyncInfo(on_wait=[w], on_update=[]),
                        )
                        nc.register_instruction(nop, overwrite=True)
                        insts.insert(i, nop)
                        i += 1
                        n += 1
                i += 1
    return n


# ---------------------------------------------------------------- program
def build_program(flags, ntiles=NT, stage=0):
    """flags: (has_bq, has_bk, has_bv, has_bp, has_b1, has_b2) bias presence."""
    has_bq, has_bk, has_bv, has_bp, has_b1, has_b2 = flags
    nc = bass.Bass()

    x_d = nc.declare_dram_parameter("x", [ntiles * P, N_EMBD], FDT, isOutput=False)
    wq_d = nc.declare_dram_parameter("wq", [128, 1024], CDT, isOutput=False)
    wk_d = nc.declare_dram_parameter("wk", [128, 1024], CDT, isOutput=False)
    wv_d = nc.declare_dram_parameter("wv", [128, 1024], CDT, isOutput=False)
    wp_d = nc.declare_dram_parameter("wp", [128, 1024], CDT, isOutput=False)
    w1_d = nc.declare_dram_parameter("w1", [128, 2048], CDT, isOutput=False)
    w2_d = nc.declare_dram_parameter("w2", [128, 2048], CDT, isOutput=False)
    id_d = nc.declare_dram_parameter("ident", [128, 128], CDT, isOutput=False)
    mk_d = nc.declare_dram_parameter("cmask", [128, T], CDT, isOutput=False)
    bq_d = bk_d = bv_d = bp_d = b1_d = b2_d = None
    if has_bq:
        bq_d = nc.declare_dram_parameter("bq", [128, 4], FDT, isOutput=False)
    if has_bk:
        bk_d = nc.declare_dram_parameter("bk", [128, 4], FDT, isOutput=False)
    if has_bv:
        bv_d = nc.declare_dram_parameter("bv", [128, 16 * 33], FDT, isOutput=False)
    if has_bp:
        bp_d = nc.declare_dram_parameter("bp", [128, N_EMBD], FDT, isOutput=False)
    if has_b1:
        b1_d = nc.declare_dram_parameter("b1", [128, 8], FDT, isOutput=False)
    if has_b2:
        b2_d = nc.declare_dram_parameter("b2", [128, N_EMBD], FDT, isOutput=False)
    out_d = nc.declare_dram_parameter("out", [ntiles * P, N_EMBD], FDT, isOutput=True)

    with tile.TileContext(nc, linearize=bool(os.environ.get('KLIN'))) as tc:
        with (
            tc.tile_pool(name="consts", bufs=1) as cpool,
            tc.tile_pool(name="work", bufs=4) as wpool,
            tc.tile_pool(name="psum", bufs=1, space="PSUM") as ppool,
        ):
            wq = cpool.tile([128, 1024], CDT)
            wk = cpool.tile([128, 1024], CDT)
            wv = cpool.tile([128, 1024], CDT)
            wp = cpool.tile([128, 1024], CDT)
            w1 = cpool.tile([128, 2048], CDT)
            w2 = cpool.tile([128, 2048], CDT)
            ident = cpool.tile([128, 128], CDT)
            cmask = cpool.tile([128, T], CDT)
            eps_sb = cpool.tile([128, 1], FDT)
            nc.gpsimd.memset(eps_sb[:], EPS)
            for t_, d_ in [(wq, wq_d), (wk, wk_d), (wv, wv_d), (wp, wp_d),
                           (w1, w1_d), (w2, w2_d), (ident, id_d), (cmask, mk_d)]:
                nc.sync.dma_start(t_[:], d_[:])
            bq = bk = bv = bpB = b1 = b2B = None
            if has_bq:
                bq = cpool.tile([128, 4], FDT)
                nc.sync.dma_start(bq[:], bq_d[:])
            if has_bk:
                bk = cpool.tile([128, 4], FDT)
                nc.sync.dma_start(bk[:], bk_d[:])
            if has_bv:
                bv = cpool.tile([128, 16 * 33], FDT)
                nc.sync.dma_start(bv[:], bv_d[:])
            if has_bp:
                bpB = cpool.tile([128, N_EMBD], FDT)
                nc.sync.dma_start(bpB[:], bp_d[:])
            if has_b1:
                b1 = cpool.tile([128, 8], FDT)
                nc.sync.dma_start(b1[:], b1_d[:])
            if has_b2:
                b2B = cpool.tile([128, N_EMBD], FDT)
                nc.sync.dma_start(b2B[:], b2_d[:])

            for it in range(ntiles):
                rows = slice(it * P, (it + 1) * P)

                # ---- load x tile (token-major [128 tok, 256 c])
                x_sb = wpool.tile([128, N_EMBD], FDT)
                nc.sync.dma_start(x_sb[:], x_d[rows, :])

                # ---- LN1 (token-major): bn stats + rstd via exp(-0.5 ln(var+eps))
                st6 = wpool.tile([128, 6], FDT, tag="st6")
                mv = wpool.tile([128, 2], FDT, tag="mv")
                if os.environ.get("KNOBN"):
                    nc.vector.memset(mv[:], 1.0)
                else:
                    nc.vector.bn_stats(st6[:], x_sb[:])
                    nc.vector.bn_aggr(mv[:], st6[:])
                lnv = wpool.tile([128, 1], FDT, tag="lnv")
                rstd = wpool.tile([128, 1], FDT, tag="rstd")
                if os.environ.get("KNOLN"):
                    nc.vector.reciprocal(rstd[:], mv[:, 1:2])
                else:
                    nc.scalar.activation(lnv[:], mv[:, 1:2], AFT.Ln, bias=eps_sb[:])
                    nc.scalar.activation(rstd[:], lnv[:], AFT.Exp, scale=-0.5)
                xhat = wpool.tile([128, N_EMBD], CDT)
                nc.vector.tensor_scalar(
                    xhat[:], x_sb[:], mv[:, 0:1], rstd[:],
                    mybir.AluOpType.subtract, mybir.AluOpType.mult,
                )

                if stage == 5:
                    out_sb = wpool.tile([128, N_EMBD], FDT)
                    nc.vector.tensor_copy(out_sb[:], xhat[:])
                    nc.sync.dma_start(out_d[rows, :], out_sb[:])
                    continue
                if stage == 6:
                    tr_ps6 = ppool.tile([128, 256], CDT, tag="tr", bufs=2)
                    for kk in range(2):
                        nc.tensor.transpose(
                            tr_ps6[:, kk * 128:(kk + 1) * 128],
                            xhat[:, kk * 128:(kk + 1) * 128], ident[:],
                        )
                    out_sb = wpool.tile([128, N_EMBD], FDT)
                    nc.vector.tensor_copy(out_sb[:], tr_ps6[:])
                    nc.sync.dma_start(out_d[rows, :], out_sb[:])
                    continue

                # ---- transpose xhat -> feature-major [c, tok] (2 chunks of 128)
                tr_ps = ppool.tile([128, 256], CDT, tag="tr", bufs=2)
                for kk in range(2):
                    nc.tensor.transpose(
                        tr_ps[:, kk * 128:(kk + 1) * 128],
                        xhat[:, kk * 128:(kk + 1) * 128], ident[:],
                    )
                xhatT = wpool.tile([128, 256], CDT)
                nc.vector.tensor_copy(xhatT[:], tr_ps[:])

                # ---- qT/kT feature-major [hd, tok]: chunk m holds heads 4m..4m+3
                q_ps = ppool.tile([128, 512], FDT, tag="mm", bufs=4)
                for m in range(4):
                    for kk in range(2):
                        nc.tensor.matmul(
                            q_ps[:, m * 128:(m + 1) * 128],
                            wq[:, kk * 512 + m * 128: kk * 512 + (m + 1) * 128],
                            xhatT[:, kk * 128:(kk + 1) * 128],
                            start=(kk == 0), stop=(kk == 1),
                        )
                qT = wpool.tile([128, 512], CDT)
                if has_bq:
                    for m in range(4):
                        nc.scalar.activation(
                            qT[:, m * 128:(m + 1) * 128],
                            q_ps[:, m * 128:(m + 1) * 128],
                            AFT.Copy, bias=0.0, scale=1.0,
                        )  # bias would need per-chunk add; use tensor_scalar below
                    # per-chunk bias add (rare path: bq nonzero)
                    for m in range(4):
                        nc.vector.tensor_scalar_add(
                            qT[:, m * 128:(m + 1) * 128],
                            qT[:, m * 128:(m + 1) * 128], bq[:, m:m + 1],
                        )
                else:
                    nc.scalar.copy(qT[:], q_ps[:])

                if stage == 7:
                    out_sb = wpool.tile([128, N_EMBD], FDT)
                    nc.vector.tensor_copy(out_sb[:], qT[:, :N_EMBD])
                    nc.sync.dma_start(out_d[rows, :], out_sb[:])
                    continue
                k_ps = ppool.tile([128, 512], FDT, tag="mm", bufs=4)
                for m in range(4):
                    for kk in range(2):
                        nc.tensor.matmul(
                            k_ps[:, m * 128:(m + 1) * 128],
                            wk[:, kk * 512 + m * 128: kk * 512 + (m + 1) * 128],
                            xhatT[:, kk * 128:(kk + 1) * 128],
                            start=(kk == 0), stop=(kk == 1),
                        )
                kT = wpool.tile([128, 512], CDT)
                if has_bk:
                    for m in range(4):
                        nc.vector.tensor_scalar_add(
                            kT[:, m * 128:(m + 1) * 128],
                            k_ps[:, m * 128:(m + 1) * 128], bk[:, m:m + 1],
                        )
                else:
                    nc.vector.tensor_copy(kT[:], k_ps[:])

                if stage == 8:
                    out_sb = wpool.tile([128, N_EMBD], FDT)
                    nc.vector.tensor_copy(out_sb[:], kT[:, :N_EMBD])
                    nc.sync.dma_start(out_d[rows, :], out_sb[:])
                    continue
                # ---- v token-major [tok, hd] with interleaved ones columns
                v_ps = ppool.tile([128, 512], FDT, tag="mm", bufs=4)
                for kk in range(2):
                    nc.tensor.matmul(
                        v_ps[:],
                        xhatT[:, kk * 128:(kk + 1) * 128],
                        wv[:, kk * 512:(kk + 1) * 512],
                        start=(kk == 0), stop=(kk == 1),
                    )
                if stage == 9:
                    out_sb = wpool.tile([128, N_EMBD], FDT)
                    nc.vector.tensor_copy(out_sb[:], v_ps[:, :N_EMBD])
                    nc.sync.dma_start(out_d[rows, :], out_sb[:])
                    continue
                v_sb = wpool.tile([128, 16 * 33], CDT)
                v_dst = v_sb[:].rearrange("p (h c) -> p h c", h=16)[:, :, 0:32]
                v_src = v_ps[:].rearrange("p (h c) -> p h c", h=16)
                if has_bv:
                    bv_ap = bv[:].rearrange("p (h c) -> p h c", h=16)[:, :, 0:32]
                    nc.vector.scalar_tensor_tensor(
                        v_dst, v_src, 1.0, bv_ap,
                        op0=mybir.AluOpType.mult, op1=mybir.AluOpType.add,
                    )
                else:
                    nc.vector.tensor_copy(v_dst, v_src)
                ones_cols = v_sb[:].rearrange("p (h c) -> p h c", h=16)[:, :, 32:33]
                if stage != 10:
                    nc.vector.memset(ones_cols, 1.0)
                if stage in (10, 11):
                    out_sb = wpool.tile([128, N_EMBD], FDT)
                    nc.vector.tensor_copy(out_sb[:], v_sb[:, :N_EMBD])
                    nc.sync.dma_start(out_d[rows, :], out_sb[:])
                    continue

                # ---- attention, 4 heads per group g
                o_sb = wpool.tile([128, 512], CDT)
                for g in range(4):
                    # scoresT[s, t] blocks: head h'=0..3 at col h'*256 (psum
                    # bank-spread); batch b at partition b*64
                    # per-head scores via prefix sums: MM over head-dims
                    # 0..32(hp+1) (row offset always 0 -- offsets 32/96 hang
                    # this HW path), then unstack by subtracting neighbors
                    sc_ps = ppool.tile([128, 256], FDT, tag="sc", bufs=2)
                    for hp in range(4):
                        for b in range(2):
                            nc.tensor.matmul(
                                sc_ps[b * 64:(b + 1) * 64,
                                      hp * 64: hp * 64 + 64],
                                kT[0:32 * (hp + 1),
                                   g * 128 + b * 64: g * 128 + (b + 1) * 64],
                                qT[0:32 * (hp + 1),
                                   g * 128 + b * 64: g * 128 + (b + 1) * 64],
                                tile_position=(0, b * 64),
                            )
                    scS = wpool.tile([128, 256], FDT, tag="scS")
                    nc.vector.tensor_copy(scS[:], sc_ps[:])
                    for hp in range(3, 0, -1):
                        nc.vector.tensor_sub(
                            scS[:, hp * 64:(hp + 1) * 64],
                            scS[:, hp * 64:(hp + 1) * 64],
                            scS[:, (hp - 1) * 64: hp * 64],
                        )
                    expT = wpool.tile([128, 256], CDT, tag="expT")
                    nc.scalar.activation(expT[:], scS[:], AFT.Exp, scale=float(SCALE))
                    # causal mask (multiplicative, broadcast over the 4 heads)
                    e_view = expT[:].rearrange("p (h s) -> p h s", h=4)
                    e_b, mk_b = bass.broadcast_tensor_aps(
                        e_view, cmask[:].rearrange("p (o s) -> p o s", o=1)
                    )
                    nc.vector.tensor_tensor(
                        e_view, e_b, mk_b, mybir.AluOpType.mult,
                    )
                    # o~[t, d] + denominator column via ones in v
                    o_ps = ppool.tile([128, 132], FDT, tag="sc", bufs=2)
                    for hp in range(4):
                        h = g * 4 + hp
                        for b in range(2):
                            nc.tensor.matmul(
                                o_ps[b * 64:(b + 1) * 64,
                                     hp * 33: hp * 33 + 33],
                                expT[b * 64:(b + 1) * 64,
                                     hp * 64:(hp + 1) * 64],
                                v_sb[b * 64:(b + 1) * 64,
                                     h * 33: h * 33 + 33],
                                tile_position=(b * 64, b * 64),
                            )
                    rec = wpool.tile([128, 4], FDT, tag="rec")
                    o_den = o_ps[:].rearrange("p (h c) -> p h c", h=4)[:, :, 32:33]
                    rec_view = rec[:].rearrange("p (h c) -> p h c", h=4)
                    nc.vector.reciprocal(rec_view, o_den)
                    o_num = o_ps[:].rearrange("p (h c) -> p h c", h=4)[:, :, 0:32]
                    o_num_b, rec_b = bass.broadcast_tensor_aps(o_num, rec_view)
                    nc.vector.tensor_tensor(
                        o_sb[:, g * 128:(g + 1) * 128].rearrange(
                            "p (h c) -> p h c", h=4),
                        o_num_b, rec_b, mybir.AluOpType.mult,
                    )

                if stage == 1:
                    out_sb = wpool.tile([128, N_EMBD], FDT)
                    nc.vector.tensor_copy(out_sb[:], qT[:, :N_EMBD])
                    nc.vector.tensor_add(out_sb[:], out_sb[:], v_sb[:, :N_EMBD])
                    nc.sync.dma_start(out_d[rows, :], out_sb[:])
                    continue
                if stage == 2:
                    out_sb = wpool.tile([128, N_EMBD], FDT)
                    nc.vector.tensor_copy(out_sb[:], o_sb[:, :N_EMBD])
                    nc.sync.dma_start(out_d[rows, :], out_sb[:])
                    continue

                # ---- transpose o -> oT feature-major (4 chunks)
                to_ps = ppool.tile([128, 512], CDT, tag="tr", bufs=2)
                for f in range(4):
                    nc.tensor.transpose(
                        to_ps[:, f * 128:(f + 1) * 128],
                        o_sb[:, f * 128:(f + 1) * 128], ident[:],
                    )
                oT = wpool.tile([128, 512], CDT)
                nc.scalar.copy(oT[:], to_ps[:])

                # ---- proj: sa = o @ Wp  (token-major out)
                sa_ps = ppool.tile([128, N_EMBD], FDT, tag="mm", bufs=4)
                for f in range(4):
                    nc.tensor.matmul(
                        sa_ps[:],
                        oT[:, f * 128:(f + 1) * 128],
                        wp[:, f * 256:(f + 1) * 256],
                        start=(f == 0), stop=(f == 3),
                    )
                # ---- residual 1
                x2_sb = wpool.tile([128, N_EMBD], FDT)
                if has_bp:
                    nc.vector.scalar_tensor_tensor(
                        x2_sb[:], sa_ps[:], 1.0, bpB[:],
                        op0=mybir.AluOpType.mult, op1=mybir.AluOpType.add,
                    )
                    nc.vector.tensor_add(x2_sb[:], x2_sb[:], x_sb[:])
                else:
                    nc.vector.tensor_add(x2_sb[:], sa_ps[:], x_sb[:])

                if stage == 3:
                    out_sb = wpool.tile([128, N_EMBD], FDT)
                    nc.vector.tensor_copy(out_sb[:], x2_sb[:])
                    nc.sync.dma_start(out_d[rows, :], out_sb[:])
                    continue

                # ---- LN2
                st6b = wpool.tile([128, 6], FDT, tag="st6")
                nc.vector.bn_stats(st6b[:], x2_sb[:])
                mvb = wpool.tile([128, 2], FDT, tag="mv")
                nc.vector.bn_aggr(mvb[:], st6b[:])
                lnvb = wpool.tile([128, 1], FDT, tag="lnv")
                nc.scalar.activation(lnvb[:], mvb[:, 1:2], AFT.Ln, bias=eps_sb[:])
                rstdb = wpool.tile([128, 1], FDT, tag="rstd")
                nc.scalar.activation(rstdb[:], lnvb[:], AFT.Exp, scale=-0.5)
                xh2 = wpool.tile([128, N_EMBD], CDT)
                nc.vector.tensor_scalar(
                    xh2[:], x2_sb[:], mvb[:, 0:1], rstdb[:],
                    mybir.AluOpType.subtract, mybir.AluOpType.mult,
                )
                t2_ps = ppool.tile([128, 256], CDT, tag="tr", bufs=2)
                for kk in range(2):
                    nc.tensor.transpose(
                        t2_ps[:, kk * 128:(kk + 1) * 128],
                        xh2[:, kk * 128:(kk + 1) * 128], ident[:],
                    )
                xh2T = wpool.tile([128, 256], CDT)
                nc.vector.tensor_copy(xh2T[:], t2_ps[:])

                # ---- FF1 feature-major: ff1T chunk m = [128 ff, 128 tok]
                rT = wpool.tile([128, 1024], CDT)
                for half in range(2):
                    f1_ps = ppool.tile([128, 512], FDT, tag="mm", bufs=4)
                    for mm_ in range(4):
                        m = half * 4 + mm_
                        for kk in range(2):
                            nc.tensor.matmul(
                                f1_ps[:, mm_ * 128:(mm_ + 1) * 128],
                                w1[:, kk * 1024 + m * 128: kk * 1024 + (m + 1) * 128],
                                xh2T[:, kk * 128:(kk + 1) * 128],
                                start=(kk == 0), stop=(kk == 1),
                            )
                    if has_b1:
                        for mm_ in range(4):
                            m = half * 4 + mm_
                            nc.scalar.activation(
                                rT[:, m * 128:(m + 1) * 128],
                                f1_ps[:, mm_ * 128:(mm_ + 1) * 128],
                                AFT.Relu, bias=b1[:, m:m + 1],
                            )
                    else:
                        nc.scalar.activation(
                            rT[:, half * 512:(half + 1) * 512],
                            f1_ps[:], AFT.Relu,
                        )

                # ---- FF2: token-major accumulate over 8 ff chunks
                ff_ps = ppool.tile([128, N_EMBD], FDT, tag="mm", bufs=4)
                for f in range(8):
                    nc.tensor.matmul(
                        ff_ps[:],
                        rT[:, f * 128:(f + 1) * 128],
                        w2[:, f * 256:(f + 1) * 256],
                        start=(f == 0), stop=(f == 7),
                    )
                # ---- residual 2 + store
                out_sb = wpool.tile([128, N_EMBD], FDT)
                nc.vector.tensor_add(out_sb[:], ff_ps[:], x2_sb[:])
                if has_b2:
                    nc.vector.tensor_add(out_sb[:], out_sb[:], b2B[:])
                nc.sync.dma_start(out_d[rows, :], out_sb[:])

    _split_waits(nc)
    nc.finalize()
    return nc


# ---------------------------------------------------------------- host prep
def _prep_weights(Wq, Wk, Wv, Wp, bp, W1, b1, W2, b2, g1, be1, g2, be2):
    import ml_dtypes

    cdt = ml_dtypes.bfloat16 if CDT == mybir.dt.bfloat16 else np.float32
    g1 = g1.astype(np.float32); be1 = be1.astype(np.float32)
    g2 = g2.astype(np.float32); be2 = be2.astype(np.float32)

    def lhsT_layout(W, n_k, n_m):  # W [K, M] -> [128, n_k * n_m * 128]
        K, M = W.shape
        return (
            W.reshape(n_k, 128, n_m, 128).transpose(1, 0, 2, 3).reshape(128, -1)
        )

    def rhs_layout(W, n_k):  # W [K, N] -> [128, n_k * N]
        K, N = W.shape
        return W.reshape(n_k, 128, N).transpose(1, 0, 2).reshape(128, -1)

    Wqf = (g1[:, None] * Wq.transpose(1, 0, 2).reshape(N_EMBD, 512)).astype(np.float32)
    Wkf = (g1[:, None] * Wk.transpose(1, 0, 2).reshape(N_EMBD, 512)).astype(np.float32)
    Wvf = (g1[:, None] * Wv.transpose(1, 0, 2).reshape(N_EMBD, 512)).astype(np.float32)
    bqv = be1 @ Wqf
    bkv = be1 @ Wkf
    bvv = be1 @ Wvf
    W1f = (g2[:, None] * W1).astype(np.float32)
    b1f = b1.astype(np.float32) + be2 @ W1f

    inp = {
        "wq": lhsT_layout(Wqf, 2, 4).astype(cdt),
        "wk": lhsT_layout(Wkf, 2, 4).astype(cdt),
        "wv": rhs_layout(Wvf, 2).astype(cdt),
        "wp": rhs_layout(Wp.astype(np.float32), 4).astype(cdt),
        "w1": lhsT_layout(W1f, 2, 8).astype(cdt),
        "w2": rhs_layout(W2.astype(np.float32), 8).astype(cdt),
        "ident": np.eye(128, dtype=np.float32).astype(cdt),
        "cmask": np.tile(
            (np.arange(T)[:, None] <= np.arange(T)[None, :]), (2, 1)
        ).astype(np.float32).astype(cdt),
    }
    flags = (
        bool(np.any(bqv)), bool(np.any(bkv)), bool(np.any(bvv)),
        bool(np.any(bp)), bool(np.any(b1f)), bool(np.any(b2)),
    )
    if flags[0]:
        inp["bq"] = bqv.reshape(4, 128).T.astype(np.float32).copy()
    if flags[1]:
        inp["bk"] = bkv.reshape(4, 128).T.astype(np.float32).copy()
    if flags[2]:
        bvt = np.zeros((128, 16 * 33), np.float32)
        bvt[:, :] = 0.0
        for h in range(N_HEAD):
            bvt[:, h * 33: h * 33 + 32] = bvv[h * 32:(h + 1) * 32][None, :]
        inp["bv"] = bvt
    if flags[3]:
        inp["bp"] = np.tile(bp.astype(np.float32)[None, :], (128, 1))
    if flags[4]:
        inp["b1"] = b1f.reshape(8, 128).T.astype(np.float32).copy()
    if flags[5]:
        inp["b2"] = np.tile(b2.astype(np.float32)[None, :], (128, 1))
    return inp, flags


_prog_cache = {}


import os
def _get_program(flags, ntiles=NT):
    stage = int(os.environ.get("KBISECT", "0"))
    key = (flags, ntiles, stage)
    if key not in _prog_cache:
        _prog_cache[key] = build_program(flags, ntiles, stage)
    return _prog_cache[key]


def run(inputs, ntiles=NT, n_cores=N_CORES, trace=False):
    """inputs: full-size dict as from setup_inputs(). Returns (out, results)."""
    x = np.asarray(inputs["x"], dtype=np.float32)
    B = x.shape[0]
    winp, flags = _prep_weights(
        *(np.asarray(inputs[k]) for k in
          ["Wq", "Wk", "Wv", "Wp", "bp", "W1", "b1", "W2", "b2",
           "g1", "be1", "g2", "be2"])
    )
    nc = _get_program(flags, ntiles)
    b_loc = B // n_cores
    shards = x.reshape(n_cores, b_loc * T, N_EMBD)
    in_maps = [dict(winp, x=np.ascontiguousarray(shards[i])) for i in range(n_cores)]
    res = run_bass_kernel_spmd(
        nc, in_maps, core_ids=list(range(n_cores)), trace=trace
    )
    out = np.concatenate(
        [res.results[i]["out"].reshape(b_loc, T, N_EMBD) for i in range(n_cores)],
        axis=0,
    )
    return out.astype(np.float32), res


def kernel(**inputs):
    out, _ = run(inputs)
    return out



# revision 11
# speedup vs baseline: 1.2345x; 1.2345x over previous
"""Trainium2 Bass kernel for a dense transformer block (pre-LN, causal MHA + FF).

Reference semantics (fp32, per batch row b of 2048, seq T=64, embd C=256):
    h   = LN(x; g1, be1)
    q,k,v = per-head projections (16 heads x 32 dims)
    att = softmax(causal(q k^T / sqrt(32))) v        -> concat heads
    x2  = x + att @ Wp + bp
    out = x2 + relu(LN(x2; g2, be2) @ W1 + b1) @ W2 + b2

Strategy: pure data parallel over 8 NeuronCores (256 batch rows each).
Per core: 128 tiles of 128 tokens (2 batch rows per tile). Token-major
layernorm (bn_stats), PE transposes to feature-major for matmuls,
per-head attention with PE tile_position packing, ones-column trick for
the softmax denominator, host-side folding of LN affine params into the
projection weights.
"""

import os
import sys

sys.path.insert(0, "/opt/trn_rl_repo")

import numpy as np
import concourse.bass as bass
import concourse.mybir as mybir
import concourse.tile as tile
from concourse.vector_clock import ScopedClock
from concourse.bass_utils import run_bass_kernel_spmd

# ---------------------------------------------------------------- constants
N_CORES = 8
N_EMBD = 256
N_HEAD = 16
HEAD = 32
T = 64
BATCH = 2048
B_LOC = BATCH // N_CORES          # 256 batch rows per core
TOK = B_LOC * T                   # 16384 tokens per core
P = 128                           # tokens per tile (2 batch rows)
NT = TOK // P                     # 128 tiles per core
SCALE = 1.0 / np.sqrt(HEAD)
EPS = 1e-5

FDT = mybir.dt.float32
# matmul operand dtype: fp8e4 streams 2 cols/cycle on the PE (measured 2x bf16)
CDT = mybir.dt.bfloat16 if os.environ.get("KBF16") else mybir.dt.float8e4
BDT = mybir.dt.bfloat16   # PE-transpose staging (fp8 transpose needs strided out)

AFT = mybir.ActivationFunctionType

# walrus (this build) only encodes 1 sync wait on CTRL-class (Drain) insts
_MAX_DRAIN_WAITS = 1


def _split_waits(nc, limit=1):
    """walrus in this build encodes only `limit` sync waits per CTRL/compute
    instruction; move overflow waits onto preceding same-engine NOPs
    (equivalent: the engine blocks at the NOP instead of at the inst).
    DMA instructions are exempt: their waits are consumed asynchronously by
    the DGE descriptor, so moving them onto a blocking sequencer NOP could
    stall the issue queue behind work that produces the awaited sem."""
    n = 0
    for f in nc.m.functions:
        for bb in f.blocks:
            insts = bb.instructions
            i = 0
            while i < len(insts):
                inst = insts[i]
                si = getattr(inst, "sync_info", None)
                if si is not None and si.on_wait and len(si.on_wait) > limit:
                    waits = list(si.on_wait)
                    keep, extra = waits[:limit], waits[limit:]
                    inst.sync_info = mybir.SyncInfo(
                        on_wait=keep, on_update=list(si.on_update or [])
                    )
                    for j, w in enumerate(extra):
                        nop = mybir.InstNoOp(
                            name=f"{inst.name}-wsplit{j}",
                            ins=[], outs=[],
                            engine=inst.engine,
                            bass_nofuse=True,
                            sync_info=mybir.SyncInfo(on_wait=[w], on_update=[]),
                        )
                        nc.register_instruction(nop, overwrite=True)
                        insts.insert(i, nop)
                        i += 1
                        n += 1
                i += 1
    return n


# ---------------------------------------------------------------- program
def build_program(flags, ntiles=NT, stage=0, reps=1):
    """flags: (has_bq, has_bk, has_bv, has_bp, has_b1, has_b2) bias presence.
    reps>1 repeats the whole tile sweep (same I/O) for steady-state timing."""
    has_bq, has_bk, has_bv, has_bp, has_b1, has_b2 = flags
    nc = bass.Bass()

    x_d = nc.declare_dram_parameter("x", [ntiles * P, N_EMBD], FDT, isOutput=False)
    wq_d = nc.declare_dram_parameter("wq", [128, 1024], CDT, isOutput=False)
    wk_d = nc.declare_dram_parameter("wk", [128, 1024], CDT, isOutput=False)
    wv_d = nc.declare_dram_parameter("wv", [128, 1024], CDT, isOutput=False)
    wp_d = nc.declare_dram_parameter("wp", [128, 1024], CDT, isOutput=False)
    w1_d = nc.declare_dram_parameter("w1", [128, 2048], CDT, isOutput=False)
    w2_d = nc.declare_dram_parameter("w2", [128, 2048], CDT, isOutput=False)
    id_d = nc.declare_dram_parameter("ident", [128, 128], BDT, isOutput=False)
    mk_d = nc.declare_dram_parameter("cmask", [128, T], CDT, isOutput=False)
    bq_d = bk_d = bv_d = bp_d = b1_d = b2_d = None
    if has_bq:
        bq_d = nc.declare_dram_parameter("bq", [128, 4], FDT, isOutput=False)
    if has_bk:
        bk_d = nc.declare_dram_parameter("bk", [128, 4], FDT, isOutput=False)
    if has_bv:
        bv_d = nc.declare_dram_parameter("bv", [128, 16 * 33], FDT, isOutput=False)
    if has_bp:
        bp_d = nc.declare_dram_parameter("bp", [128, N_EMBD], FDT, isOutput=False)
    if has_b1:
        b1_d = nc.declare_dram_parameter("b1", [128, 8], FDT, isOutput=False)
    if has_b2:
        b2_d = nc.declare_dram_parameter("b2", [128, N_EMBD], FDT, isOutput=False)
    out_d = nc.declare_dram_parameter("out", [ntiles * P, N_EMBD], FDT, isOutput=True)

    with tile.TileContext(nc, linearize=bool(os.environ.get('KLIN'))) as tc:
        with (
            tc.tile_pool(name="consts", bufs=1) as cpool,
            tc.tile_pool(name="work", bufs=int(os.environ.get("KWBUFS", "6"))) as wpool,
            tc.tile_pool(name="psum", bufs=1, space="PSUM") as ppool,
        ):
            wq = cpool.tile([128, 1024], CDT)
            wk = cpool.tile([128, 1024], CDT)
            wv = cpool.tile([128, 1024], CDT)
            wp = cpool.tile([128, 1024], CDT)
            w1 = cpool.tile([128, 2048], CDT)
            w2 = cpool.tile([128, 2048], CDT)
            ident = cpool.tile([128, 128], BDT)
            cmask = cpool.tile([128, T], CDT)
            eps_sb = cpool.tile([128, 1], FDT)
            nc.gpsimd.memset(eps_sb[:], EPS)
            for t_, d_ in [(wq, wq_d), (wk, wk_d), (wv, wv_d), (wp, wp_d),
                           (w1, w1_d), (w2, w2_d), (ident, id_d), (cmask, mk_d)]:
                nc.sync.dma_start(t_[:], d_[:])
            bq = bk = bv = bpB = b1 = b2B = None
            if has_bq:
                bq = cpool.tile([128, 4], FDT)
                nc.sync.dma_start(bq[:], bq_d[:])
            if has_bk:
                bk = cpool.tile([128, 4], FDT)
                nc.sync.dma_start(bk[:], bk_d[:])
            if has_bv:
                bv = cpool.tile([128, 16 * 33], FDT)
                nc.sync.dma_start(bv[:], bv_d[:])
            if has_bp:
                bpB = cpool.tile([128, N_EMBD], FDT)
                nc.sync.dma_start(bpB[:], bp_d[:])
            if has_b1:
                b1 = cpool.tile([128, 8], FDT)
                nc.sync.dma_start(b1[:], b1_d[:])
            if has_b2:
                b2B = cpool.tile([128, N_EMBD], FDT)
                nc.sync.dma_start(b2B[:], b2_d[:])

            for it_ in range(ntiles * reps):
                it = it_ % ntiles
                rows = slice(it * P, (it + 1) * P)

                # ---- load x tile (token-major [128 tok, 256 c])
                x_sb = wpool.tile([128, N_EMBD], FDT)
                nc.sync.dma_start(x_sb[:], x_d[rows, :])

                # ---- LN1 (token-major): bn stats + rstd via exp(-0.5 ln(var+eps))
                st6 = wpool.tile([128, 6], FDT, tag="st6")
                mv = wpool.tile([128, 2], FDT, tag="mv")
                if os.environ.get("KNOBN"):
                    nc.vector.memset(mv[:], 1.0)
                else:
                    nc.vector.bn_stats(st6[:], x_sb[:])
                    nc.vector.bn_aggr(mv[:], st6[:])
                lnv = wpool.tile([128, 1], FDT, tag="lnv")
                rstd = wpool.tile([128, 1], FDT, tag="rstd")
                if os.environ.get("KNOLN"):
                    nc.vector.reciprocal(rstd[:], mv[:, 1:2])
                else:
                    nc.scalar.activation(lnv[:], mv[:, 1:2], AFT.Ln, bias=eps_sb[:])
                    nc.scalar.activation(rstd[:], lnv[:], AFT.Exp, scale=-0.5)
                xhat = wpool.tile([128, N_EMBD], BDT)
                nc.vector.tensor_scalar(
                    xhat[:], x_sb[:], mv[:, 0:1], rstd[:],
                    mybir.AluOpType.subtract, mybir.AluOpType.mult,
                )

                if stage == 5:
                    out_sb = wpool.tile([128, N_EMBD], FDT)
                    nc.vector.tensor_copy(out_sb[:], xhat[:])
                    nc.sync.dma_start(out_d[rows, :], out_sb[:])
                    continue
                if stage == 6:
                    tr_ps6 = ppool.tile([128, 256], BDT, tag="tr", bufs=2)
                    for kk in range(2):
                        nc.tensor.transpose(
                            tr_ps6[:, kk * 128:(kk + 1) * 128],
                            xhat[:, kk * 128:(kk + 1) * 128], ident[:],
                        )
                    out_sb = wpool.tile([128, N_EMBD], FDT)
                    nc.vector.tensor_copy(out_sb[:], tr_ps6[:])
                    nc.sync.dma_start(out_d[rows, :], out_sb[:])
                    continue

                # ---- transpose xhat -> feature-major [c, tok] (2 chunks of 128)
                tr_ps = ppool.tile([128, 256], BDT, tag="tr", bufs=2)
                for kk in range(2):
                    nc.tensor.transpose(
                        tr_ps[:, kk * 128:(kk + 1) * 128],
                        xhat[:, kk * 128:(kk + 1) * 128], ident[:],
                    )
                xhatT = wpool.tile([128, 256], CDT)
                nc.vector.tensor_copy(xhatT[:], tr_ps[:])

                # ---- qT/kT feature-major [hd, tok]: chunk m holds heads 4m..4m+3
                q_ps = ppool.tile([128, 512], FDT, tag="mm", bufs=3)
                for m in range(4):
                    for kk in range(2):
                        nc.tensor.matmul(
                            q_ps[:, m * 128:(m + 1) * 128],
                            wq[:, kk * 512 + m * 128: kk * 512 + (m + 1) * 128],
                            xhatT[:, kk * 128:(kk + 1) * 128],
                            start=(kk == 0), stop=(kk == 1),
                        )
                qT = wpool.tile([128, 512], CDT)
                if has_bq:
                    for m in range(4):
                        nc.scalar.activation(
                            qT[:, m * 128:(m + 1) * 128],
                            q_ps[:, m * 128:(m + 1) * 128],
                            AFT.Copy, bias=0.0, scale=1.0,
                        )  # bias would need per-chunk add; use tensor_scalar below
                    # per-chunk bias add (rare path: bq nonzero)
                    for m in range(4):
                        nc.vector.tensor_scalar_add(
                            qT[:, m * 128:(m + 1) * 128],
                            qT[:, m * 128:(m + 1) * 128], bq[:, m:m + 1],
                        )
                else:
                    nc.scalar.copy(qT[:], q_ps[:])

                if stage == 7:
                    out_sb = wpool.tile([128, N_EMBD], FDT)
                    nc.vector.tensor_copy(out_sb[:], qT[:, :N_EMBD])
                    nc.sync.dma_start(out_d[rows, :], out_sb[:])
                    continue
                k_ps = ppool.tile([128, 512], FDT, tag="mm", bufs=3)
                for m in range(4):
                    for kk in range(2):
                        nc.tensor.matmul(
                            k_ps[:, m * 128:(m + 1) * 128],
                            wk[:, kk * 512 + m * 128: kk * 512 + (m + 1) * 128],
                            xhatT[:, kk * 128:(kk + 1) * 128],
                            start=(kk == 0), stop=(kk == 1),
                        )
                kT = wpool.tile([128, 512], CDT)
                if has_bk:
                    for m in range(4):
                        nc.vector.tensor_scalar_add(
                            kT[:, m * 128:(m + 1) * 128],
                            k_ps[:, m * 128:(m + 1) * 128], bk[:, m:m + 1],
                        )
                else:
                    nc.vector.tensor_copy(kT[:], k_ps[:])

                if stage == 8:
                    out_sb = wpool.tile([128, N_EMBD], FDT)
                    nc.vector.tensor_copy(out_sb[:], kT[:, :N_EMBD])
                    nc.sync.dma_start(out_d[rows, :], out_sb[:])
                    continue
                # ---- v token-major [tok, hd] with interleaved ones columns
                v_ps = ppool.tile([128, 512], FDT, tag="mm", bufs=3)
                for kk in range(2):
                    nc.tensor.matmul(
                        v_ps[:],
                        xhatT[:, kk * 128:(kk + 1) * 128],
                        wv[:, kk * 512:(kk + 1) * 512],
                        start=(kk == 0), stop=(kk == 1),
                    )
                if stage == 9:
                    out_sb = wpool.tile([128, N_EMBD], FDT)
                    nc.vector.tensor_copy(out_sb[:], v_ps[:, :N_EMBD])
                    nc.sync.dma_start(out_d[rows, :], out_sb[:])
                    continue
                v_sb = wpool.tile([128, 16 * 33], CDT)
                v_dst = v_sb[:].rearrange("p (h c) -> p h c", h=16)[:, :, 0:32]
                v_src = v_ps[:].rearrange("p (h c) -> p h c", h=16)
                if has_bv:
                    bv_ap = bv[:].rearrange("p (h c) -> p h c", h=16)[:, :, 0:32]
                    nc.vector.scalar_tensor_tensor(
                        v_dst, v_src, 1.0, bv_ap,
                        op0=mybir.AluOpType.mult, op1=mybir.AluOpType.add,
                    )
                else:
                    nc.vector.tensor_copy(v_dst, v_src)
                ones_cols = v_sb[:].rearrange("p (h c) -> p h c", h=16)[:, :, 32:33]
                if stage != 10:
                    nc.vector.memset(ones_cols, 1.0)
                if stage in (10, 11):
                    out_sb = wpool.tile([128, N_EMBD], FDT)
                    nc.vector.tensor_copy(out_sb[:], v_sb[:, :N_EMBD])
                    nc.sync.dma_start(out_d[rows, :], out_sb[:])
                    continue

                # ---- attention, 4 heads per group g
                o_sb = wpool.tile([128, 512], BDT)
                for g in range(4):
                    # scoresT[s, t] blocks: head h'=0..3 at col h'*256 (psum
                    # bank-spread); batch b at partition b*64
                    # per-head scores via prefix sums: MM over head-dims
                    # 0..32(hp+1) (row offset always 0 -- offsets 32/96 hang
                    # this HW path), then unstack by subtracting neighbors
                    sc_ps = ppool.tile([128, 256], FDT, tag="sc", bufs=2)
                    for hp in range(4):
                        for b in range(2):
                            nc.tensor.matmul(
                                sc_ps[b * 64:(b + 1) * 64,
                                      hp * 64: hp * 64 + 64],
                                kT[0:32 * (hp + 1),
                                   g * 128 + b * 64: g * 128 + (b + 1) * 64],
                                qT[0:32 * (hp + 1),
                                   g * 128 + b * 64: g * 128 + (b + 1) * 64],
                                tile_position=(0, b * 64),
                            )
                    scS = wpool.tile([128, 256], FDT, tag="scS")
                    nc.vector.tensor_copy(scS[:], sc_ps[:])
                    for hp in range(3, 0, -1):
                        nc.vector.tensor_sub(
                            scS[:, hp * 64:(hp + 1) * 64],
                            scS[:, hp * 64:(hp + 1) * 64],
                            scS[:, (hp - 1) * 64: hp * 64],
                        )
                    expT = wpool.tile([128, 256], CDT, tag="expT")
                    nc.scalar.activation(expT[:], scS[:], AFT.Exp, scale=float(SCALE))
                    # causal mask (multiplicative, broadcast over the 4 heads)
                    e_view = expT[:].rearrange("p (h s) -> p h s", h=4)
                    e_b, mk_b = bass.broadcast_tensor_aps(
                        e_view, cmask[:].rearrange("p (o s) -> p o s", o=1)
                    )
                    nc.vector.tensor_tensor(
                        e_view, e_b, mk_b, mybir.AluOpType.mult,
                    )
                    # o~[t, d] + denominator column via ones in v
                    o_ps = ppool.tile([128, 132], FDT, tag="sco", bufs=1)
                    for hp in range(4):
                        h = g * 4 + hp
                        for b in range(2):
                            nc.tensor.matmul(
                                o_ps[b * 64:(b + 1) * 64,
                                     hp * 33: hp * 33 + 33],
                                expT[b * 64:(b + 1) * 64,
                                     hp * 64:(hp + 1) * 64],
                                v_sb[b * 64:(b + 1) * 64,
                                     h * 33: h * 33 + 33],
                                tile_position=(b * 64, b * 64),
                            )
                    rec = wpool.tile([128, 4], FDT, tag="rec")
                    o_den = o_ps[:].rearrange("p (h c) -> p h c", h=4)[:, :, 32:33]
                    rec_view = rec[:].rearrange("p (h c) -> p h c", h=4)
                    nc.vector.reciprocal(rec_view, o_den)
                    o_num = o_ps[:].rearrange("p (h c) -> p h c", h=4)[:, :, 0:32]
                    o_num_b, rec_b = bass.broadcast_tensor_aps(o_num, rec_view)
                    nc.vector.tensor_tensor(
                        o_sb[:, g * 128:(g + 1) * 128].rearrange(
                            "p (h c) -> p h c", h=4),
                        o_num_b, rec_b, mybir.AluOpType.mult,
                    )

                if stage == 1:
                    out_sb = wpool.tile([128, N_EMBD], FDT)
                    nc.vector.tensor_copy(out_sb[:], qT[:, :N_EMBD])
                    nc.vector.tensor_add(out_sb[:], out_sb[:], v_sb[:, :N_EMBD])
                    nc.sync.dma_start(out_d[rows, :], out_sb[:])
                    continue
                if stage == 2:
                    out_sb = wpool.tile([128, N_EMBD], FDT)
                    nc.vector.tensor_copy(out_sb[:], o_sb[:, :N_EMBD])
                    nc.sync.dma_start(out_d[rows, :], out_sb[:])
                    continue

                # ---- transpose o -> oT feature-major (4 chunks)
                to_ps = ppool.tile([128, 512], BDT, tag="tr", bufs=2)
                for f in range(4):
                    nc.tensor.transpose(
                        to_ps[:, f * 128:(f + 1) * 128],
                        o_sb[:, f * 128:(f + 1) * 128], ident[:],
                    )
                oT = wpool.tile([128, 512], CDT)
                nc.scalar.copy(oT[:], to_ps[:])

                # ---- proj: sa = o @ Wp  (token-major out)
                sa_ps = ppool.tile([128, N_EMBD], FDT, tag="mm", bufs=3)
                for f in range(4):
                    nc.tensor.matmul(
                        sa_ps[:],
                        oT[:, f * 128:(f + 1) * 128],
                        wp[:, f * 256:(f + 1) * 256],
                        start=(f == 0), stop=(f == 3),
                    )
                # ---- residual 1
                x2_sb = wpool.tile([128, N_EMBD], FDT)
                if has_bp:
                    nc.vector.scalar_tensor_tensor(
                        x2_sb[:], sa_ps[:], 1.0, bpB[:],
                        op0=mybir.AluOpType.mult, op1=mybir.AluOpType.add,
                    )
                    nc.vector.tensor_add(x2_sb[:], x2_sb[:], x_sb[:])
                else:
                    nc.vector.tensor_add(x2_sb[:], sa_ps[:], x_sb[:])

                if stage == 3:
                    out_sb = wpool.tile([128, N_EMBD], FDT)
                    nc.vector.tensor_copy(out_sb[:], x2_sb[:])
                    nc.sync.dma_start(out_d[rows, :], out_sb[:])
                    continue

                # ---- LN2
                st6b = wpool.tile([128, 6], FDT, tag="st6")
                nc.vector.bn_stats(st6b[:], x2_sb[:])
                mvb = wpool.tile([128, 2], FDT, tag="mv")
                nc.vector.bn_aggr(mvb[:], st6b[:])
                lnvb = wpool.tile([128, 1], FDT, tag="lnv")
                nc.scalar.activation(lnvb[:], mvb[:, 1:2], AFT.Ln, bias=eps_sb[:])
                rstdb = wpool.tile([128, 1], FDT, tag="rstd")
                nc.scalar.activation(rstdb[:], lnvb[:], AFT.Exp, scale=-0.5)
                xh2 = wpool.tile([128, N_EMBD], BDT)
                nc.vector.tensor_scalar(
                    xh2[:], x2_sb[:], mvb[:, 0:1], rstdb[:],
                    mybir.AluOpType.subtract, mybir.AluOpType.mult,
                )
                t2_ps = ppool.tile([128, 256], BDT, tag="tr", bufs=2)
                for kk in range(2):
                    nc.tensor.transpose(
                        t2_ps[:, kk * 128:(kk + 1) * 128],
                        xh2[:, kk * 128:(kk + 1) * 128], ident[:],
                    )
                xh2T = wpool.tile([128, 256], CDT)
                nc.vector.tensor_copy(xh2T[:], t2_ps[:])

                # ---- FF1 feature-major: ff1T chunk m = [128 ff, 128 tok]
                rT = wpool.tile([128, 1024], CDT)
                for half in range(2):
                    f1_ps = ppool.tile([128, 512], FDT, tag="mm", bufs=3)
                    for mm_ in range(4):
                        m = half * 4 + mm_
                        for kk in range(2):
                            nc.tensor.matmul(
                                f1_ps[:, mm_ * 128:(mm_ + 1) * 128],
                                w1[:, kk * 1024 + m * 128: kk * 1024 + (m + 1) * 128],
                                xh2T[:, kk * 128:(kk + 1) * 128],
                                start=(kk == 0), stop=(kk == 1),
                            )
                    if has_b1:
                        for mm_ in range(4):
                            m = half * 4 + mm_
                            nc.scalar.activation(
                                rT[:, m * 128:(m + 1) * 128],
                                f1_ps[:, mm_ * 128:(mm_ + 1) * 128],
                                AFT.Relu, bias=b1[:, m:m + 1],
                            )
                    else:
                        nc.scalar.activation(
                            rT[:, half * 512:(half + 1) * 512],
                            f1_ps[:], AFT.Relu,
                        )

                # ---- FF2: token-major accumulate over 8 ff chunks
                ff_ps = ppool.tile([128, N_EMBD], FDT, tag="mm", bufs=3)
                for f in range(8):
                    nc.tensor.matmul(
                        ff_ps[:],
                        rT[:, f * 128:(f + 1) * 128],
                        w2[:, f * 256:(f + 1) * 256],
                        start=(f == 0), stop=(f == 7),
                    )
                # ---- residual 2 + store
                out_sb = wpool.tile([128, N_EMBD], FDT)
                nc.vector.tensor_add(out_sb[:], ff_ps[:], x2_sb[:])
                if has_b2:
                    nc.vector.tensor_add(out_sb[:], out_sb[:], b2B[:])
                nc.sync.dma_start(out_d[rows, :], out_sb[:])

    _split_waits(nc)
    nc.finalize()
    return nc


# ---------------------------------------------------------------- host prep
def _prep_weights(Wq, Wk, Wv, Wp, bp, W1, b1, W2, b2, g1, be1, g2, be2):
    cdt = mybir.dt.np(CDT)
    g1 = g1.astype(np.float32); be1 = be1.astype(np.float32)
    g2 = g2.astype(np.float32); be2 = be2.astype(np.float32)

    def lhsT_layout(W, n_k, n_m):  # W [K, M] -> [128, n_k * n_m * 128]
        K, M = W.shape
        return (
            W.reshape(n_k, 128, n_m, 128).transpose(1, 0, 2, 3).reshape(128, -1)
        )

    def rhs_layout(W, n_k):  # W [K, N] -> [128, n_k * N]
        K, N = W.shape
        return W.reshape(n_k, 128, N).transpose(1, 0, 2).reshape(128, -1)

    Wqf = (g1[:, None] * Wq.transpose(1, 0, 2).reshape(N_EMBD, 512)).astype(np.float32)
    Wkf = (g1[:, None] * Wk.transpose(1, 0, 2).reshape(N_EMBD, 512)).astype(np.float32)
    Wvf = (g1[:, None] * Wv.transpose(1, 0, 2).reshape(N_EMBD, 512)).astype(np.float32)
    bqv = be1 @ Wqf
    bkv = be1 @ Wkf
    bvv = be1 @ Wvf
    W1f = (g2[:, None] * W1).astype(np.float32)
    b1f = b1.astype(np.float32) + be2 @ W1f

    inp = {
        "wq": lhsT_layout(Wqf, 2, 4).astype(cdt),
        "wk": lhsT_layout(Wkf, 2, 4).astype(cdt),
        "wv": rhs_layout(Wvf, 2).astype(cdt),
        "wp": rhs_layout(Wp.astype(np.float32), 4).astype(cdt),
        "w1": lhsT_layout(W1f, 2, 8).astype(cdt),
        "w2": rhs_layout(W2.astype(np.float32), 8).astype(cdt),
        "ident": np.eye(128, dtype=np.float32).astype(mybir.dt.np(BDT)),
        "cmask": np.tile(
            (np.arange(T)[:, None] <= np.arange(T)[None, :]), (2, 1)
        ).astype(np.float32).astype(cdt),
    }
    flags = (
        bool(np.any(bqv)), bool(np.any(bkv)), bool(np.any(bvv)),
        bool(np.any(bp)), bool(np.any(b1f)), bool(np.any(b2)),
    )
    if flags[0]:
        inp["bq"] = bqv.reshape(4, 128).T.astype(np.float32).copy()
    if flags[1]:
        inp["bk"] = bkv.reshape(4, 128).T.astype(np.float32).copy()
    if flags[2]:
        bvt = np.zeros((128, 16 * 33), np.float32)
        bvt[:, :] = 0.0
        for h in range(N_HEAD):
            bvt[:, h * 33: h * 33 + 32] = bvv[h * 32:(h + 1) * 32][None, :]
        inp["bv"] = bvt
    if flags[3]:
        inp["bp"] = np.tile(bp.astype(np.float32)[None, :], (128, 1))
    if flags[4]:
        inp["b1"] = b1f.reshape(8, 128).T.astype(np.float32).copy()
    if flags[5]:
        inp["b2"] = np.tile(b2.astype(np.float32)[None, :], (128, 1))
    return inp, flags


_prog_cache = {}


import os
def _get_program(flags, ntiles=NT):
    stage = int(os.environ.get("KBISECT", "0"))
    reps = int(os.environ.get("KREPS", "1"))
    key = (flags, ntiles, stage, reps)
    if key not in _prog_cache:
        _prog_cache[key] = build_program(flags, ntiles, stage, reps)
    return _prog_cache[key]


def run(inputs, ntiles=NT, n_cores=N_CORES, trace=False):
    """inputs: full-size dict as from setup_inputs(). Returns (out, results)."""
    x = np.asarray(inputs["x"], dtype=np.float32)
    B = x.shape[0]
    winp, flags = _prep_weights(
        *(np.asarray(inputs[k]) for k in
          ["Wq", "Wk", "Wv", "Wp", "bp", "W1", "b1", "W2", "b2",
           "g1", "be1", "g2", "be2"])
    )
    nc = _get_program(flags, ntiles)
    b_loc = B // n_cores
    shards = x.reshape(n_cores, b_loc * T, N_EMBD)
    in_maps = [dict(winp, x=np.ascontiguousarray(shards[i])) for i in range(n_cores)]
    res = run_bass_kernel_spmd(
        nc, in_maps, core_ids=list(range(n_cores)), trace=trace
    )
    out = np.concatenate(
        [res.results[i]["out"].reshape(b_loc, T, N_EMBD) for i in range(n_cores)],
        axis=0,
    )
    return out.astype(np.float32), res


def kernel(**inputs):
    out, _ = run(inputs)
    return out

